# revision 10
# baseline (speedup 1.0000x reference)
"""Trainium2 Bass kernel for sparse (top-k) multi-headed attention.

Problem shapes (hardcoded):
  x, source: [B=4, D=256, N=M=2048] f32
  Wq/Wk/Wv/Wm: [256, 256], bq/bk/bv/bm: [256], k=32 (top-k), H=4 heads, dim=64.

Sharding: 8 cores; core c handles batch b=c//2 and head pair hp=c%2
(heads 2hp, 2hp+1).  Channel c of D maps to (d, h) = (c//4, c%4) per the
reference reshape(B, dim, H, N).  The host reorders each core's 128
channels head-major/d-major so each head occupies 64 contiguous SBUF
partitions.  Each core returns its partial merge
  part = Wm[:, ch].T? -> out_part[o, n] = sum_{i in ch} Wm[o, i] * merged[i, n]
and the host sums the two partials per batch and adds bm.

v2 per-tile pipeline (tile = 128 query rows x 2048 keys, 32 tiles/core):
  PE   : 4 fp32 score matmuls -> one [128,2048] PSUM tile (4 banks)
  ACT  : s_sb = copy(PSUM) ; e = exp(scale*s_sb) in bf16 (reads SBUF so
         PSUM is freed after the copy)
  DVE  : top-k peel = 4x max8 + 3x match_replace (the last match_replace
         is unnecessary: m32 is sorted, t = m32[:,k-1] is the threshold)
  ACT  : e32 = exp(scale*m32) with accum_out=den (one tiny instruction)
  DVE  : rden = 1/den
  GPS  : pn = (s_sb >= t) * e        (scalar_tensor_tensor, bf16 out)
  GPS  : diag = identity_bf * rden   (per-tile diagonal normalizer)
  PE   : pT = pn_slice.T @ diag      (transpose matmul folds the softmax
         normalization in for free), then AV accumulation in bf16
  fp32r (4x faster PE) for the v-projection and final merge (both are
  post-selection, so reduced precision cannot perturb the top-k set).
"""

import os
import sys

import ml_dtypes
import numpy as np

for _p in ("/opt/trn_rl_repo",):
    if _p not in sys.path and os.path.isdir(_p):
        sys.path.insert(0, _p)

import concourse.bass as bass
import concourse.mybir as mybir
import concourse.tile as tile
from concourse.bass_utils import run_bass_kernel_spmd
from concourse.masks import make_identity

B, D, N, M = 4, 256, 2048, 2048
H = 4
DIM = D // H  # 64
P = 128
NT = N // P  # n-tiles of 128 rows
MT = M // P  # m-tiles of 128 cols
SCALE = 1.0 / float(np.sqrt(DIM))  # 0.125
N_CORES = 8

FP = mybir.dt.float32
FR = mybir.dt.float32r
BF = mybir.dt.bfloat16
A = mybir.AluOpType
AF = mybir.ActivationFunctionType


def _legalize_sync_waits(bir: dict) -> dict:
    """Split multi-wait instructions: walrus codegen allows only ONE sync wait
    per engine instruction (PE is HW-decoded; ACT/CTRL structs are just as
    limited).  Insert single-wait NoOps on the same engine immediately before
    any instruction carrying more than one wait; each NoOp takes one wait, the
    original keeps the last wait plus its updates."""
    nid = [0]
    for fn in bir["functions"]:
        for blk in fn["blocks"]:
            out = []
            for ins in blk["instructions"]:
                si = ins.get("sync_info")
                waits = (si or {}).get("on_wait") or []
                if len(waits) > 1:
                    for w in waits[:-1]:
                        nid[0] += 1
                        out.append(
                            {
                                "engine": ins["engine"],
                                "ins": [],
                                "name": f"{ins['name']}-sw{nid[0]}",
                                "opcode": "NoOp",
                                "outs": [],
                                "sync_info": {"on_update": [], "on_wait": [w]},
                            }
                        )
                    si["on_wait"] = [waits[-1]]
                out.append(ins)
            blk["instructions"] = out
    return bir


def build_program(k: int) -> bass.Bass:
    nrounds = (k + 7) // 8
    rem = k - (nrounds - 1) * 8  # valid slots in the last round (1..8)

    nc = bass.Bass(
        "TRN2",
        target_bir_lowering=False,
        debug=False,
        enable_asserts=True,
        num_devices=N_CORES,
    )

    # DRAM parameters (per-core shards, prepared by the host)
    xb = nc.dram_tensor("xb", [D, N], FP, kind="ExternalInput").ap()
    src = nc.dram_tensor("src", [D, M], FP, kind="ExternalInput").ap()
    wqT = nc.dram_tensor("wqT", [D, P], FP, kind="ExternalInput").ap()
    wkT = nc.dram_tensor("wkT", [D, P], FP, kind="ExternalInput").ap()
    wvT = nc.dram_tensor("wvT", [D, P], FP, kind="ExternalInput").ap()
    wmT = nc.dram_tensor("wmT", [P, D], FP, kind="ExternalInput").ap()
    bqv = nc.dram_tensor("bq", [P, 1], FP, kind="ExternalInput").ap()
    bkv = nc.dram_tensor("bk", [P, 1], FP, kind="ExternalInput").ap()
    bvv = nc.dram_tensor("bv", [P, 1], FP, kind="ExternalInput").ap()
    part = nc.dram_tensor("part", [D, N], FP, kind="ExternalOutput").ap()

    from contextlib import ExitStack

    with tile.TileContext(nc) as tc, ExitStack() as ctx:
        consts = ctx.enter_context(tc.tile_pool(name="consts", bufs=1))
        wpool = ctx.enter_context(tc.tile_pool(name="w", bufs=1))
        qkvp = ctx.enter_context(tc.tile_pool(name="qkv", bufs=1))
        vtp = ctx.enter_context(tc.tile_pool(name="vt", bufs=1))
        xpool_cm = tc.tile_pool(name="x", bufs=1)
        xpool = xpool_cm.__enter__()

        identity = consts.tile([P, P], FP)
        make_identity(nc, identity)

        # ---- load weights / biases / activations ----
        w_tiles = {}
        for name, ap in (("wq", wqT), ("wk", wkT), ("wv", wvT)):
            t0 = wpool.tile([P, P], FP, tag=name + "0")
            t1 = wpool.tile([P, P], FP, tag=name + "1")
            nc.sync.dma_start(out=t0[:], in_=ap[0:P, :])
            nc.sync.dma_start(out=t1[:], in_=ap[P : 2 * P, :])
            w_tiles[name] = (t0, t1)
        wm_sb = wpool.tile([P, D], FP, tag="wm")
        nc.sync.dma_start(out=wm_sb[:], in_=wmT[:, :])
        b_tiles = {}
        for name, ap in (("bq", bqv), ("bk", bkv), ("bv", bvv)):
            t = wpool.tile([P, 1], FP, tag=name)
            nc.sync.dma_start(out=t[:], in_=ap[:, :])
            b_tiles[name] = t

        x_sb = [xpool.tile([P, N], FP, tag=f"x{i}", name=f"x{i}") for i in range(2)]
        src_sb = [xpool.tile([P, M], FP, tag=f"s{i}", name=f"s{i}") for i in range(2)]
        for i in range(2):
            nc.sync.dma_start(out=x_sb[i][:], in_=xb[i * P : (i + 1) * P, :])
            nc.sync.dma_start(out=src_sb[i][:], in_=src[i * P : (i + 1) * P, :])

        # Persistent PSUM pools (8 banks total, never recycled across phases):
        #   sps: one [P, 2048] fp32 score tile  = 4 banks
        #   tps: junk / vT transposes / p-transposes (bufs=2)
        #   avps: AV accumulator [64, 512]      = 1 bank
        #   mgps: merge matmul [128, 512]       = 1 bank
        sps = ctx.enter_context(tc.tile_pool(name="sps", bufs=1, space="PSUM"))
        tps = ctx.enter_context(tc.tile_pool(name="tps", bufs=2, space="PSUM"))
        avps = ctx.enter_context(tc.tile_pool(name="avps", bufs=1, space="PSUM"))
        mgps = ctx.enter_context(tc.tile_pool(name="mgps", bufs=1, space="PSUM"))

        # Absorb every DMA-completion semaphore (and the gpsimd-built
        # identity) into PE's observed clock: one tiny single-wait matmul per
        # loaded tile, so no later PE instruction needs a second fresh wait.
        all_loaded = (
            [w_tiles[n][i] for n in ("wq", "wk", "wv") for i in range(2)]
            + [wm_sb]
            + [b_tiles[n] for n in ("bq", "bk", "bv")]
            + x_sb
            + src_sb
            + [identity]
        )
        junk = tps.tile([P, P], FP, tag="ptp", name="junk")
        for i, t in enumerate(all_loaded):
            nc.tensor.matmul(
                junk[0:1, i : i + 1], lhsT=t[:, 0:1], rhs=t[:, 0:1],
                start=True, stop=True, skip_group_check=True,
            )
        # Read the junk tile on ACT so any later PE instruction recycling this
        # PSUM slot waits on the Activation sem (its one allowed wait).
        junk_sink = consts.tile([1, len(all_loaded)], FP, name="junk_sink")
        nc.scalar.activation(
            out=junk_sink[:], in_=junk[0:1, 0 : len(all_loaded)],
            func=AF.Copy,
        )

        # ---- QKV projections: out[ch, n] = sum_i wT[i, ch] * in[i, n] + b[ch]
        # All three write through the single [P, 2048] score PSUM tile's four
        # banks round-robin.  v runs in fp32r (post-selection, 4x faster).
        q_sb = qkvp.tile([P, N], FP, tag="q")
        k_sb = qkvp.tile([P, M], FP, tag="k")
        v_sb = qkvp.tile([P, M], FP, tag="v")
        qkv_ps = sps.tile([P, M], FP, tag="sc", name="qkvps")
        for (wname, bname, ins, out_sb) in (
            ("wk", "bk", src_sb, k_sb),
            ("wq", "bq", x_sb, q_sb),
            ("wv", "bv", src_sb, v_sb),
        ):
            w0, w1 = w_tiles[wname]
            bt = b_tiles[bname]
            for nf in range(4):
                sl = slice(nf * 512, (nf + 1) * 512)
                pp = qkv_ps[:, sl]
                nc.tensor.matmul(
                    pp, lhsT=w0[:], rhs=ins[0][:, sl],
                    start=True, stop=False, skip_group_check=True,
                )
                nc.tensor.matmul(
                    pp, lhsT=w1[:], rhs=ins[1][:, sl],
                    start=False, stop=True, skip_group_check=True,
                )
                nc.scalar.activation(
                    out=out_sb[:, sl], in_=pp,
                    func=AF.Identity, bias=bt[:],
                )

        xpool_cm.__exit__(None, None, None)

        # ---- vT per head (bf16): vT_sb[h] cols mt*64.. = v_h[:, mt*128..].T
        vT_sb = [
            vtp.tile([P, MT * DIM], BF, tag=f"vT{h}", name=f"vT{h}") for h in range(2)
        ]
        for h in range(2):
            hs = slice(h * DIM, (h + 1) * DIM)
            for mt in range(MT):
                tp = tps.tile([P, P], FP, tag="ptp", name="vtp")
                nc.tensor.transpose(
                    tp[0:P, 0:DIM], v_sb[hs, mt * P : (mt + 1) * P], identity[hs, hs]
                )
                nc.scalar.activation(
                    out=vT_sb[h][:, mt * DIM : (mt + 1) * DIM], in_=tp[0:P, 0:DIM],
                    func=AF.Copy,
                )

        # ---- main loop: stage-skewed software pipeline over 32 tiles ----
        # slot t stages: scores(t) -> peel(t) -> den(t-1) -> rden/ln/e(t-2)
        #   -> select(t-3, GpSimd) -> transpose-DMA(t-4) -> AV chunks (PE)
        scp = ctx.enter_context(tc.tile_pool(name="ssb", bufs=5))
        srp = ctx.enter_context(tc.tile_pool(name="scr", bufs=2))
        epool = ctx.enter_context(tc.tile_pool(name="e", bufs=3))
        mkp = ctx.enter_context(tc.tile_pool(name="mk", bufs=2))
        pnp = ctx.enter_context(tc.tile_pool(name="pn", bufs=3))
        ptp = ctx.enter_context(tc.tile_pool(name="pt", bufs=2))
        m32p = ctx.enter_context(tc.tile_pool(name="m32", bufs=6))
        dpool = ctx.enter_context(tc.tile_pool(name="den", bufs=4))
        mgp = ctx.enter_context(tc.tile_pool(name="mg", bufs=2))
        NEG = -1.0e30
        ST = 4  # n-tiles per super-tile (phase)

        phases = [(st, h) for st in range(NT // ST) for h in range(2)]
        NTILES = len(phases) * ST
        tstate = {}  # per-tile tiles for the skewed stages
        pstate = {}  # per-phase: pTph tile, av psum tile
        mgs = {}  # st -> mg_sb tile
        av_fifo = []

        def st_scores(t):
            p, ntl = t // ST, t % ST
            st, h = phases[p]
            hs = slice(h * DIM, (h + 1) * DIM)
            nn0 = (st * ST + ntl) * P
            sp = sps.tile([P, M], FP, tag="sc", name="sp")
            for mf in range(4):
                sl = slice(mf * 512, (mf + 1) * 512)
                nc.tensor.matmul(
                    sp[:, sl], lhsT=q_sb[hs, nn0 : nn0 + P], rhs=k_sb[hs, sl],
                    start=True, stop=True, skip_group_check=True,
                )
            s_sb = scp.tile([P, M], FP, tag="ssb", name="s_sb")
            nc.scalar.activation(out=s_sb[:], in_=sp[:], func=AF.Copy)
            tstate[t] = dict(s_sb=s_sb)

        def st_peel(t):
            # top-k peel on raw fp32 scores: 4x max8 + 3x match_replace
            s_sb = tstate[t]["s_sb"]
            scratch = srp.tile([P, M], FP, tag="scratch", name="scratch")
            m32 = m32p.tile([P, 8 * nrounds], FP, tag="m32", name="m32")
            src_t = s_sb
            for r in range(nrounds):
                m8 = m32[:, r * 8 : (r + 1) * 8]
                nc.vector.max(out=m8, in_=src_t[:])
                if r == nrounds - 1 and rem < 8:
                    nc.vector.memset(m8[:, rem:], NEG)
                if r < nrounds - 1:
                    nc.vector.match_replace(
                        out=scratch[:], in_to_replace=m8, in_values=src_t[:],
                        imm_value=NEG,
                    )
                    src_t = scratch
            tstate[t]["m32"] = m32

        def st_den(t):
            # ACT: den = sum(exp(scale * m32)) via accumulate output
            stt = tstate[t]
            e32 = dpool.tile([P, 8 * nrounds], FP, tag="e32", name="e32")
            den = dpool.tile([P, 1], FP, tag="den", name="den")
            nc.scalar.activation(
                out=e32[:], in_=stt["m32"][:], func=AF.Exp, scale=float(SCALE),
                accum_out=den[:],
            )
            stt["den"] = den

        def st_recip(t):
            stt = tstate[t]
            rden = dpool.tile([P, 1], FP, tag="rden", name="rden")
            nc.vector.reciprocal(rden[:], stt["den"][:])
            stt["rden"] = rden

        def st_exp(t):
            # nlden = ln(1/den); e = exp(scale*s - ln den) = softmax numerator/den
            stt = tstate[t]
            nlden = dpool.tile([P, 1], FP, tag="nlden", name="nlden")
            nc.scalar.activation(out=nlden[:], in_=stt["rden"][:], func=AF.Ln)
            e_sb = epool.tile([P, M], BF, tag="e", name="e")
            nc.scalar.activation(
                out=e_sb[:], in_=stt["s_sb"][:], func=AF.Exp, scale=float(SCALE),
                bias=nlden[:],
            )
            stt["e_sb"] = e_sb

        def st_mask(t):
            # DVE: mask = (s >= t_k) in bf16 (exact fp32 compare)
            stt = tstate[t]
            mask = mkp.tile([P, M], BF, tag="mask", name="mask")
            nc.vector.tensor_scalar(
                mask[:], stt["s_sb"][:], stt["m32"][:, k - 1 : k], None, A.is_ge
            )
            stt["mask"] = mask

        def st_select(t):
            # GpSimd: pn = mask * e, already normalized by den
            stt = tstate[t]
            pn = pnp.tile([P, M], BF, tag="pn", name="pn")
            nc.gpsimd.tensor_tensor(pn[:], stt["mask"][:], stt["e_sb"][:], A.mult)
            stt["pn"] = pn

        def st_dmat(t):
            # transpose-DMA pn into the per-phase pT layout [m_local, mt, n]
            p, ntl = t // ST, t % ST
            if ntl == 0:
                pstate[p] = dict(
                    pT=ptp.tile([P, MT * ST * P], BF, tag="pT", name="pT")
                )
            pT = pstate[p]["pT"]
            pn = tstate[t]["pn"]
            for mt in range(MT):
                nc.sync.dma_start_transpose(
                    out=pT[:, mt * ST * P + ntl * P : mt * ST * P + (ntl + 1) * P],
                    in_=pn[:, mt * P : (mt + 1) * P],
                )
            if ntl == ST - 1:
                av_fifo.extend((p, c) for c in range(ST))

        def st_av_chunk():
            if not av_fifo:
                return
            p, c = av_fifo.pop(0)
            st, h = phases[p]
            ps = pstate[p]
            if c == 0:
                ps["av"] = avps.tile([DIM, ST * P], FP, tag="av", name="av")
            for mt in range(c * 4, c * 4 + 4):
                nc.tensor.matmul(
                    ps["av"][:], lhsT=vT_sb[h][:, mt * DIM : (mt + 1) * DIM],
                    rhs=ps["pT"][:, mt * ST * P : (mt + 1) * ST * P],
                    start=(mt == 0), stop=(mt == MT - 1),
                    skip_group_check=True,
                )
            if c == ST - 1:
                _phase_end(p)

        def _phase_end(p):
            # av -> mg_sb; on odd phases also the merge matmul + output DMA
            st, h = phases[p]
            hs = slice(h * DIM, (h + 1) * DIM)
            if h == 0:
                mgs[st] = mgp.tile([P, ST * P], FP, tag="mg", name="mg")
            mg_sb = mgs[st]
            nc.scalar.activation(out=mg_sb[hs, :], in_=pstate[p]["av"][:], func=AF.Copy)
            if h == 1:
                n0 = st * ST * P
                for oh in range(2):
                    mm = mgps.tile([P, ST * P], FP, tag="mm", name="mm")
                    nc.tensor.matmul(
                        mm[:], lhsT=wm_sb[:, oh * P : (oh + 1) * P], rhs=mg_sb[:],
                        start=True, stop=True, skip_group_check=True,
                    )
                    mo = mgp.tile([P, ST * P], FP, tag="mo", name="mo")
                    nc.scalar.activation(out=mo[:], in_=mm[:], func=AF.Copy)
                    nc.sync.dma_start(
                        out=part[oh * P : (oh + 1) * P, n0 : n0 + ST * P], in_=mo[:]
                    )

        for t in range(NTILES + 9):
            if t < NTILES:
                st_scores(t)
            if 0 <= t - 2 < NTILES:
                st_recip(t - 2)
                st_mask(t - 2)
            if t < NTILES:
                st_peel(t)
            if 0 <= t - 1 < NTILES:
                st_den(t - 1)
            if 0 <= t - 2 < NTILES:
                st_exp(t - 2)
            if 0 <= t - 3 < NTILES:
                st_select(t - 3)
            if 0 <= t - 4 < NTILES:
                st_dmat(t - 4)
            st_av_chunk()

    import json as _json

    d = _json.loads(nc.to_json_bytes())
    _legalize_sync_waits(d)
    blob = _json.dumps(d).encode()
    nc.to_json_bytes = lambda: blob  # shadow the method; bass2jax serializes via this
    return nc


_PROGRAM_CACHE: dict[int, object] = {}
LAST_RESULTS = None


def _channel_order(hp: int) -> list[int]:
    # head-major, d-major within head: channels of head h are {4d + h}
    return [4 * d + 2 * hp + j for j in (0, 1) for d in range(DIM)]


def make_in_maps(x, source, Wq, bq, Wk, bk, Wv, bv, Wm):
    in_maps = []
    for c in range(N_CORES):
        b = c // 2
        hp = c % 2
        ch = _channel_order(hp)
        in_maps.append(
            {
                "xb": np.ascontiguousarray(x[b], dtype=np.float32),
                "src": np.ascontiguousarray(source[b], dtype=np.float32),
                "wqT": np.ascontiguousarray(Wq[ch, :].T, dtype=np.float32),
                "wkT": np.ascontiguousarray(Wk[ch, :].T, dtype=np.float32),
                "wvT": np.ascontiguousarray(Wv[ch, :].T, dtype=np.float32),
                "wmT": np.ascontiguousarray(Wm[:, ch].T, dtype=np.float32),
                "bq": np.ascontiguousarray(bq[ch].reshape(P, 1), dtype=np.float32),
                "bk": np.ascontiguousarray(bk[ch].reshape(P, 1), dtype=np.float32),
                "bv": np.ascontiguousarray(bv[ch].reshape(P, 1), dtype=np.float32),
            }
        )
    return in_maps


class _CompiledProgram:
    """Builds the Bass program once and caches the jitted shard_map callable
    (mirrors the multi-core branch of bass2jax.run_bass_via_pjrt)."""

    def __init__(self, k: int):
        import jax
        from jax.sharding import Mesh, PartitionSpec
        from jax.experimental.shard_map import shard_map
        from concourse import bass2jax

        bass2jax.install_neuronx_cc_hook()
        nc = build_program(k)
        self.nc = nc
        import concourse.mybir as _mybir

        in_names, out_names, out_avals, zero_outs = [], [], [], []
        for alloc in nc.m.functions[0].allocations:
            if not isinstance(alloc, _mybir.MemoryLocationSet):
                continue
            name = alloc.memorylocations[0].name
            partition_name = (
                nc.partition_id_tensor.name if nc.partition_id_tensor else None
            )
            if alloc.kind == "ExternalInput":
                if name != partition_name:
                    in_names.append(name)
            elif alloc.kind == "ExternalOutput":
                out_names.append(name)
                shape = tuple(alloc.tensor_shape)
                dtype = _mybir.dt.np(alloc.dtype)
                out_avals.append(jax.core.ShapedArray(shape, dtype))
                zero_outs.append(np.zeros(shape, dtype))
        self.in_names = list(in_names)
        self.out_names = out_names
        n_params = len(in_names)
        n_outs = len(out_avals)
        in_names = in_names + out_names
        self.in_names = self.in_names[:n_params]
        donate = tuple(range(n_params, n_params + n_outs))
        self.zero_outs = zero_outs
        self.out_avals = out_avals

        partition_name = (
            nc.partition_id_tensor.name if nc.partition_id_tensor else None
        )
        if partition_name is not None:
            in_names = in_names + [partition_name]

        def _body(*args):
            operands = list(args)
            if partition_name is not None:
                operands.append(bass2jax.partition_id_tensor())
            outs = bass2jax._bass_exec_p.bind(
                *operands,
                out_avals=tuple(out_avals),
                in_names=tuple(in_names),
                out_names=tuple(out_names),
                lowering_input_output_aliases=(),
                sim_require_finite=True,
                sim_require_nnan=True,
                nc=nc,
            )
            return tuple(outs)

        devices = jax.devices()[:N_CORES]
        mesh = Mesh(np.asarray(devices), ("core",))
        in_specs = (PartitionSpec("core"),) * (n_params + n_outs)
        out_specs = (PartitionSpec("core"),) * len(out_names)
        self.sharded = jax.jit(
            shard_map(
                _body, mesh=mesh, in_specs=in_specs, out_specs=out_specs,
                check_rep=False,
            ),
            donate_argnums=donate,
            keep_unused=True,
        )
        self.jax = jax

    def run(self, in_maps):
        np_in = [
            np.concatenate([np.asarray(m[name]) for m in in_maps], axis=0)
            for name in self.in_names
        ]
        zeros = [
            np.zeros((N_CORES * z.shape[0], *z.shape[1:]), z.dtype)
            for z in self.zero_outs
        ]
        out_arrs = self.jax.block_until_ready(self.sharded(*np_in, *zeros))
        return [
            {
                name: np.asarray(out_arrs[i]).reshape(
                    N_CORES, *self.out_avals[i].shape
                )[c]
                for i, name in enumerate(self.out_names)
            }
            for c in range(N_CORES)
        ]


def _get_program(k: int) -> _CompiledProgram:
    prog = _PROGRAM_CACHE.get(k)
    if prog is None:
        prog = _CompiledProgram(k)
        _PROGRAM_CACHE[k] = prog
    return prog


def kernel(x, source, Wq, bq, Wk, bk, Wv, bv, Wm, bm, k):
    global LAST_RESULTS
    k = int(k)
    x = np.asarray(x, dtype=np.float32)
    source = np.asarray(source, dtype=np.float32)
    prog = _get_program(k)
    in_maps = make_in_maps(x, source, Wq, bq, Wk, bk, Wv, bv, Wm)
    results = prog.run(in_maps)
    LAST_RESULTS = results
    out = np.zeros((B, D, N), dtype=np.float32)
    for c in range(N_CORES):
        out[c // 2] += results[c]["part"]
    out += np.asarray(bm, dtype=np.float32)[None, :, None]
    return out


# revision 15
# speedup vs baseline: 1.2646x; 1.2646x over previous
"""Trainium2 Bass kernel for sparse (top-k) multi-headed attention.

Problem shapes (hardcoded):
  x, source: [B=4, D=256, N=M=2048] f32
  Wq/Wk/Wv/Wm: [256, 256], bq/bk/bv/bm: [256], k=32 (top-k), H=4 heads, dim=64.

Sharding: 8 cores; core c handles batch b=c//2 and head pair hp=c%2
(heads 2hp, 2hp+1).  Channel c of D maps to (d, h) = (c//4, c%4) per the
reference reshape(B, dim, H, N).  The host reorders each core's 128
channels head-major/d-major so each head occupies 64 contiguous SBUF
partitions.  Each core returns its partial merge
  part = Wm[:, ch].T? -> out_part[o, n] = sum_{i in ch} Wm[o, i] * merged[i, n]
and the host sums the two partials per batch and adds bm.

v2 per-tile pipeline (tile = 128 query rows x 2048 keys, 32 tiles/core):
  PE   : 4 fp32 score matmuls -> one [128,2048] PSUM tile (4 banks)
  ACT  : s_sb = copy(PSUM) ; e = exp(scale*s_sb) in bf16 (reads SBUF so
         PSUM is freed after the copy)
  DVE  : top-k peel = 4x max8 + 3x match_replace (the last match_replace
         is unnecessary: m32 is sorted, t = m32[:,k-1] is the threshold)
  ACT  : e32 = exp(scale*m32) with accum_out=den (one tiny instruction)
  DVE  : rden = 1/den
  GPS  : pn = (s_sb >= t) * e        (scalar_tensor_tensor, bf16 out)
  GPS  : diag = identity_bf * rden   (per-tile diagonal normalizer)
  PE   : pT = pn_slice.T @ diag      (transpose matmul folds the softmax
         normalization in for free), then AV accumulation in bf16
  fp32r (4x faster PE) for the v-projection and final merge (both are
  post-selection, so reduced precision cannot perturb the top-k set).
"""

import os
import sys

import ml_dtypes
import numpy as np

for _p in ("/opt/trn_rl_repo",):
    if _p not in sys.path and os.path.isdir(_p):
        sys.path.insert(0, _p)

import concourse.bass as bass
import concourse.mybir as mybir
import concourse.tile as tile
from concourse.bass_utils import run_bass_kernel_spmd
from concourse.masks import make_identity

B, D, N, M = 4, 256, 2048, 2048
H = 4
DIM = D // H  # 64
P = 128
NT = N // P  # n-tiles of 128 rows
MT = M // P  # m-tiles of 128 cols
SCALE = 1.0 / float(np.sqrt(DIM))  # 0.125
N_CORES = 8

FP = mybir.dt.float32
FR = mybir.dt.float32r
BF = mybir.dt.bfloat16
A = mybir.AluOpType
AF = mybir.ActivationFunctionType


def _legalize_sync_waits(bir: dict) -> dict:
    """Split multi-wait instructions: walrus codegen allows only ONE sync wait
    per engine instruction (PE is HW-decoded; ACT/CTRL structs are just as
    limited).  Insert single-wait NoOps on the same engine immediately before
    any instruction carrying more than one wait; each NoOp takes one wait, the
    original keeps the last wait plus its updates."""
    nid = [0]
    for fn in bir["functions"]:
        for blk in fn["blocks"]:
            out = []
            for ins in blk["instructions"]:
                si = ins.get("sync_info")
                waits = (si or {}).get("on_wait") or []
                if len(waits) > 1:
                    for w in waits[:-1]:
                        nid[0] += 1
                        out.append(
                            {
                                "engine": ins["engine"],
                                "ins": [],
                                "name": f"{ins['name']}-sw{nid[0]}",
                                "opcode": "NoOp",
                                "outs": [],
                                "sync_info": {"on_update": [], "on_wait": [w]},
                            }
                        )
                    si["on_wait"] = [waits[-1]]
                out.append(ins)
            blk["instructions"] = out
    return bir


def build_program(k: int) -> bass.Bass:
    nrounds = (k + 7) // 8
    rem = k - (nrounds - 1) * 8  # valid slots in the last round (1..8)

    nc = bass.Bass(
        "TRN2",
        target_bir_lowering=False,
        debug=False,
        enable_asserts=True,
        num_devices=N_CORES,
    )

    # DRAM parameters (per-core shards, prepared by the host)
    xb = nc.dram_tensor("xb", [D, N], FP, kind="ExternalInput").ap()
    src = nc.dram_tensor("src", [D, M], FP, kind="ExternalInput").ap()
    wqT = nc.dram_tensor("wqT", [D, P], FP, kind="ExternalInput").ap()
    wkT = nc.dram_tensor("wkT", [D, P], FP, kind="ExternalInput").ap()
    wvT = nc.dram_tensor("wvT", [D, P], FP, kind="ExternalInput").ap()
    wmT = nc.dram_tensor("wmT", [P, D], FP, kind="ExternalInput").ap()
    bqv = nc.dram_tensor("bq", [P, 1], FP, kind="ExternalInput").ap()
    bkv = nc.dram_tensor("bk", [P, 1], FP, kind="ExternalInput").ap()
    bvv = nc.dram_tensor("bv", [P, 1], FP, kind="ExternalInput").ap()
    part = nc.dram_tensor("part", [D, N], FP, kind="ExternalOutput").ap()

    from contextlib import ExitStack

    with tile.TileContext(nc) as tc, ExitStack() as ctx:
        consts = ctx.enter_context(tc.tile_pool(name="consts", bufs=1))
        wpool = ctx.enter_context(tc.tile_pool(name="w", bufs=1))
        qkvp = ctx.enter_context(tc.tile_pool(name="qkv", bufs=1))
        vtp = ctx.enter_context(tc.tile_pool(name="vt", bufs=1))
        xpool_cm = tc.tile_pool(name="x", bufs=1)
        xpool = xpool_cm.__enter__()

        identity = consts.tile([P, P], FP)
        make_identity(nc, identity)

        # ---- load weights / biases / activations ----
        w_tiles = {}
        for name, ap in (("wq", wqT), ("wk", wkT), ("wv", wvT)):
            t0 = wpool.tile([P, P], FP, tag=name + "0")
            t1 = wpool.tile([P, P], FP, tag=name + "1")
            nc.sync.dma_start(out=t0[:], in_=ap[0:P, :])
            nc.sync.dma_start(out=t1[:], in_=ap[P : 2 * P, :])
            w_tiles[name] = (t0, t1)
        wm_sb = wpool.tile([P, D], FP, tag="wm")
        nc.sync.dma_start(out=wm_sb[:], in_=wmT[:, :])
        b_tiles = {}
        for name, ap in (("bq", bqv), ("bk", bkv), ("bv", bvv)):
            t = wpool.tile([P, 1], FP, tag=name)
            nc.sync.dma_start(out=t[:], in_=ap[:, :])
            b_tiles[name] = t

        x_sb = [xpool.tile([P, N], FP, tag=f"x{i}", name=f"x{i}") for i in range(2)]
        src_sb = [xpool.tile([P, M], FP, tag=f"s{i}", name=f"s{i}") for i in range(2)]
        for i in range(2):
            nc.sync.dma_start(out=x_sb[i][:], in_=xb[i * P : (i + 1) * P, :])
            nc.sync.dma_start(out=src_sb[i][:], in_=src[i * P : (i + 1) * P, :])

        # Persistent PSUM pools (8 banks total, never recycled across phases):
        #   sps: one [P, 2048] fp32 score tile  = 4 banks
        #   tps: junk / vT transposes / p-transposes (bufs=2)
        #   avps: AV accumulator [64, 512]      = 1 bank
        #   mgps: merge matmul [128, 512]       = 1 bank
        sps = ctx.enter_context(tc.tile_pool(name="sps", bufs=1, space="PSUM"))
        tps = ctx.enter_context(tc.tile_pool(name="tps", bufs=2, space="PSUM"))
        avps = ctx.enter_context(tc.tile_pool(name="avps", bufs=1, space="PSUM"))
        mgps = ctx.enter_context(tc.tile_pool(name="mgps", bufs=1, space="PSUM"))

        # Absorb every DMA-completion semaphore (and the gpsimd-built
        # identity) into PE's observed clock: one tiny single-wait matmul per
        # loaded tile, so no later PE instruction needs a second fresh wait.
        all_loaded = (
            [w_tiles[n][i] for n in ("wq", "wk", "wv") for i in range(2)]
            + [wm_sb]
            + [b_tiles[n] for n in ("bq", "bk", "bv")]
            + x_sb
            + src_sb
            + [identity]
        )
        junk = tps.tile([P, P], FP, tag="ptp", name="junk")
        for i, t in enumerate(all_loaded):
            nc.tensor.matmul(
                junk[0:1, i : i + 1], lhsT=t[:, 0:1], rhs=t[:, 0:1],
                start=True, stop=True, skip_group_check=True,
            )
        # Read the junk tile on ACT so any later PE instruction recycling this
        # PSUM slot waits on the Activation sem (its one allowed wait).
        junk_sink = consts.tile([1, len(all_loaded)], FP, name="junk_sink")
        nc.scalar.activation(
            out=junk_sink[:], in_=junk[0:1, 0 : len(all_loaded)],
            func=AF.Copy,
        )

        # ---- QKV projections: out[ch, n] = sum_i wT[i, ch] * in[i, n] + b[ch]
        # All three write through the single [P, 2048] score PSUM tile's four
        # banks round-robin.  v runs in fp32r (post-selection, 4x faster).
        q_sb = qkvp.tile([P, N], FP, tag="q")
        k_sb = qkvp.tile([P, M], FP, tag="k")
        v_sb = qkvp.tile([P, M], FP, tag="v")
        qkv_ps = sps.tile([P, M], FP, tag="sc", name="qkvps")
        for (wname, bname, ins, out_sb) in (
            ("wk", "bk", src_sb, k_sb),
            ("wq", "bq", x_sb, q_sb),
            ("wv", "bv", src_sb, v_sb),
        ):
            w0, w1 = w_tiles[wname]
            bt = b_tiles[bname]
            for nf in range(4):
                sl = slice(nf * 512, (nf + 1) * 512)
                pp = qkv_ps[:, sl]
                nc.tensor.matmul(
                    pp, lhsT=w0[:], rhs=ins[0][:, sl],
                    start=True, stop=False, skip_group_check=True,
                )
                nc.tensor.matmul(
                    pp, lhsT=w1[:], rhs=ins[1][:, sl],
                    start=False, stop=True, skip_group_check=True,
                )
                nc.scalar.activation(
                    out=out_sb[:, sl], in_=pp,
                    func=AF.Identity, bias=bt[:],
                )

        xpool_cm.__exit__(None, None, None)

        # ---- vT per head (bf16): vT_sb[h] cols mt*64.. = v_h[:, mt*128..].T
        vT_sb = [
            vtp.tile([P, MT * DIM], BF, tag=f"vT{h}", name=f"vT{h}") for h in range(2)
        ]
        for h in range(2):
            hs = slice(h * DIM, (h + 1) * DIM)
            for mt in range(MT):
                tp = tps.tile([P, P], FP, tag="ptp", name="vtp")
                nc.tensor.transpose(
                    tp[0:P, 0:DIM], v_sb[hs, mt * P : (mt + 1) * P], identity[hs, hs]
                )
                nc.scalar.activation(
                    out=vT_sb[h][:, mt * DIM : (mt + 1) * DIM], in_=tp[0:P, 0:DIM],
                    func=AF.Copy,
                )

        # ---- main loop: stage-skewed software pipeline over 32 tiles ----
        # slot t stages: scores(t) -> peel(t) -> den(t-1) -> rden/ln/e(t-2)
        #   -> select(t-3, GpSimd) -> transpose-DMA(t-4) -> AV chunks (PE)
        scp = ctx.enter_context(tc.tile_pool(name="ssb", bufs=5))
        srp = ctx.enter_context(tc.tile_pool(name="scr", bufs=2))
        epool = ctx.enter_context(tc.tile_pool(name="e", bufs=3))
        mkp = ctx.enter_context(tc.tile_pool(name="mk", bufs=2))
        pnp = ctx.enter_context(tc.tile_pool(name="pn", bufs=3))
        ptp = ctx.enter_context(tc.tile_pool(name="pt", bufs=2))
        m32p = ctx.enter_context(tc.tile_pool(name="m32", bufs=6))
        dpool = ctx.enter_context(tc.tile_pool(name="den", bufs=4))
        mgp = ctx.enter_context(tc.tile_pool(name="mg", bufs=2))
        NEG = -1.0e30
        ST = 4  # n-tiles per super-tile (phase)

        phases = [(st, h) for st in range(NT // ST) for h in range(2)]
        NTILES = len(phases) * ST
        tstate = {}  # per-tile tiles for the skewed stages
        pstate = {}  # per-phase: pTph tile, av psum tile
        mgs = {}  # st -> mg_sb tile
        av_fifo = []

        def st_scores(t):
            p, ntl = t // ST, t % ST
            st, h = phases[p]
            hs = slice(h * DIM, (h + 1) * DIM)
            nn0 = (st * ST + ntl) * P
            sp = sps.tile([P, M], FP, tag="sc", name="sp")
            for mf in range(4):
                sl = slice(mf * 512, (mf + 1) * 512)
                nc.tensor.matmul(
                    sp[:, sl], lhsT=q_sb[hs, nn0 : nn0 + P], rhs=k_sb[hs, sl],
                    start=True, stop=True, skip_group_check=True,
                )
            s_sb = scp.tile([P, M], FP, tag="ssb", name="s_sb")
            nc.scalar.activation(out=s_sb[:], in_=sp[:], func=AF.Copy)
            e_sb = epool.tile([P, M], BF, tag="e", name="e")
            nc.scalar.activation(
                out=e_sb[:], in_=s_sb[:], func=AF.Exp, scale=float(SCALE)
            )
            tstate[t] = dict(s_sb=s_sb, e_sb=e_sb)

        def st_peel(t):
            # top-k peel on raw fp32 scores: 4x max8 + 3x match_replace
            s_sb = tstate[t]["s_sb"]
            scratch = srp.tile([P, M], FP, tag="scratch", name="scratch")
            m32 = m32p.tile([P, 8 * nrounds], FP, tag="m32", name="m32")
            src_t = s_sb
            for r in range(nrounds):
                m8 = m32[:, r * 8 : (r + 1) * 8]
                nc.vector.max(out=m8, in_=src_t[:])
                if r == nrounds - 1 and rem < 8:
                    nc.vector.memset(m8[:, rem:], NEG)
                if r < nrounds - 1:
                    nc.vector.match_replace(
                        out=scratch[:], in_to_replace=m8, in_values=src_t[:],
                        imm_value=NEG,
                    )
                    src_t = scratch
            tstate[t]["m32"] = m32

        def st_den(t):
            # ACT: den = sum(exp(scale * m32)) via accumulate output
            stt = tstate[t]
            e32 = dpool.tile([P, 8 * nrounds], FP, tag="e32", name="e32")
            den = dpool.tile([P, 1], FP, tag="den", name="den")
            nc.scalar.activation(
                out=e32[:], in_=stt["m32"][:], func=AF.Exp, scale=float(SCALE),
                accum_out=den[:],
            )
            stt["den"] = den

        def st_recip(t):
            stt = tstate[t]
            rden = dpool.tile([P, 1], FP, tag="rden", name="rden")
            nc.vector.reciprocal(rden[:], stt["den"][:])
            stt["rden"] = rden

        def st_mask(t):
            # DVE: mask = (s >= t_k) * rden in bf16 (exact fp32 compare, with
            # the softmax normalizer folded in; exp LUT args stay unshifted so
            # numerator/denominator LUT values cancel exactly)
            stt = tstate[t]
            mask = mkp.tile([P, M], BF, tag="mask", name="mask")
            nc.vector.tensor_scalar(
                mask[:], stt["s_sb"][:], stt["m32"][:, k - 1 : k], stt["rden"][:],
                A.is_ge, A.mult,
            )
            stt["mask"] = mask

        def st_select(t):
            # GpSimd: pn = mask * e, already normalized by den
            stt = tstate[t]
            pn = pnp.tile([P, M], BF, tag="pn", name="pn")
            nc.gpsimd.tensor_tensor(pn[:], stt["mask"][:], stt["e_sb"][:], A.mult)
            stt["pn"] = pn

        def st_dmat(t):
            # One batched transpose-DMA of the whole [128, 2048] pn tile into
            # the per-phase pT layout [m_local, mt, n]; alternate the issue
            # queue between Sync and Activation to halve queue pressure.
            p, ntl = t // ST, t % ST
            if ntl == 0:
                pstate[p] = dict(
                    pT=ptp.tile([P, MT, ST * P], BF, tag="pT", name="pT")
                )
            pT = pstate[p]["pT"]
            pn = tstate[t]["pn"]
            eng = nc.sync if t % 2 == 0 else nc.scalar
            eng.dma_start_transpose(
                out=pT[:, :, ntl * P : (ntl + 1) * P],
                in_=pn[:],
            )
            if ntl == ST - 1:
                av_fifo.extend((p, c) for c in range(ST))

        def st_av_chunk():
            if not av_fifo:
                return
            p, c = av_fifo.pop(0)
            st, h = phases[p]
            ps = pstate[p]
            if c == 0:
                ps["av"] = avps.tile([DIM, ST * P], FP, tag="av", name="av")
            for mt in range(c * 4, c * 4 + 4):
                nc.tensor.matmul(
                    ps["av"][:], lhsT=vT_sb[h][:, mt * DIM : (mt + 1) * DIM],
                    rhs=ps["pT"][:, mt, :],
                    start=(mt == 0), stop=(mt == MT - 1),
                    skip_group_check=True,
                )
            if c == ST - 1:
                _phase_end(p)

        def _phase_end(p):
            # av -> mg_sb; on odd phases also the merge matmul + output DMA
            st, h = phases[p]
            hs = slice(h * DIM, (h + 1) * DIM)
            if h == 0:
                mgs[st] = mgp.tile([P, ST * P], FP, tag="mg", name="mg")
            mg_sb = mgs[st]
            nc.scalar.activation(out=mg_sb[hs, :], in_=pstate[p]["av"][:], func=AF.Copy)
            if h == 1:
                n0 = st * ST * P
                for oh in range(2):
                    mm = mgps.tile([P, ST * P], FP, tag="mm", name="mm")
                    nc.tensor.matmul(
                        mm[:], lhsT=wm_sb[:, oh * P : (oh + 1) * P], rhs=mg_sb[:],
                        start=True, stop=True, skip_group_check=True,
                    )
                    mo = mgp.tile([P, ST * P], FP, tag="mo", name="mo")
                    nc.scalar.activation(out=mo[:], in_=mm[:], func=AF.Copy)
                    nc.sync.dma_start(
                        out=part[oh * P : (oh + 1) * P, n0 : n0 + ST * P], in_=mo[:]
                    )

        for t in range(NTILES + 9):
            if t < NTILES:
                st_scores(t)
            if 0 <= t - 2 < NTILES:
                st_recip(t - 2)
                st_mask(t - 2)
            if t < NTILES:
                st_peel(t)
            if 0 <= t - 1 < NTILES:
                st_den(t - 1)
            if 0 <= t - 3 < NTILES:
                st_select(t - 3)
            if 0 <= t - 4 < NTILES:
                st_dmat(t - 4)
            st_av_chunk()

    import json as _json

    d = _json.loads(nc.to_json_bytes())
    _legalize_sync_waits(d)
    blob = _json.dumps(d).encode()
    nc.to_json_bytes = lambda: blob  # shadow the method; bass2jax serializes via this
    return nc


_PROGRAM_CACHE: dict[int, object] = {}
LAST_RESULTS = None


def _channel_order(hp: int) -> list[int]:
    # head-major, d-major within head: channels of head h are {4d + h}
    return [4 * d + 2 * hp + j for j in (0, 1) for d in range(DIM)]


def make_in_maps(x, source, Wq, bq, Wk, bk, Wv, bv, Wm):
    in_maps = []
    for c in range(N_CORES):
        b = c // 2
        hp = c % 2
        ch = _channel_order(hp)
        in_maps.append(
            {
                "xb": np.ascontiguousarray(x[b], dtype=np.float32),
                "src": np.ascontiguousarray(source[b], dtype=np.float32),
                "wqT": np.ascontiguousarray(Wq[ch, :].T, dtype=np.float32),
                "wkT": np.ascontiguousarray(Wk[ch, :].T, dtype=np.float32),
                "wvT": np.ascontiguousarray(Wv[ch, :].T, dtype=np.float32),
                "wmT": np.ascontiguousarray(Wm[:, ch].T, dtype=np.float32),
                "bq": np.ascontiguousarray(bq[ch].reshape(P, 1), dtype=np.float32),
                "bk": np.ascontiguousarray(bk[ch].reshape(P, 1), dtype=np.float32),
                "bv": np.ascontiguousarray(bv[ch].reshape(P, 1), dtype=np.float32),
            }
        )
    return in_maps


class _CompiledProgram:
    """Builds the Bass program once and caches the jitted shard_map callable
    (mirrors the multi-core branch of bass2jax.run_bass_via_pjrt)."""

    def __init__(self, k: int):
        import jax
        from jax.sharding import Mesh, PartitionSpec
        from jax.experimental.shard_map import shard_map
        from concourse import bass2jax

        bass2jax.install_neuronx_cc_hook()
        nc = build_program(k)
        self.nc = nc
        import concourse.mybir as _mybir

        in_names, out_names, out_avals, zero_outs = [], [], [], []
        for alloc in nc.m.functions[0].allocations:
            if not isinstance(alloc, _mybir.MemoryLocationSet):
                continue
            name = alloc.memorylocations[0].name
            partition_name = (
                nc.partition_id_tensor.name if nc.partition_id_tensor else None
            )
            if alloc.kind == "ExternalInput":
                if name != partition_name:
                    in_names.append(name)
            elif alloc.kind == "ExternalOutput":
                out_names.append(name)
                shape = tuple(alloc.tensor_shape)
                dtype = _mybir.dt.np(alloc.dtype)
                out_avals.append(jax.core.ShapedArray(shape, dtype))
                zero_outs.append(np.zeros(shape, dtype))
        self.in_names = list(in_names)
        self.out_names = out_names
        n_params = len(in_names)
        n_outs = len(out_avals)
        in_names = in_names + out_names
        self.in_names = self.in_names[:n_params]
        donate = tuple(range(n_params, n_params + n_outs))
        self.zero_outs = zero_outs
        self.out_avals = out_avals

        partition_name = (
            nc.partition_id_tensor.name if nc.partition_id_tensor else None
        )
        if partition_name is not None:
            in_names = in_names + [partition_name]

        def _body(*args):
            operands = list(args)
            if partition_name is not None:
                operands.append(bass2jax.partition_id_tensor())
            outs = bass2jax._bass_exec_p.bind(
                *operands,
                out_avals=tuple(out_avals),
                in_names=tuple(in_names),
                out_names=tuple(out_names),
                lowering_input_output_aliases=(),
                sim_require_finite=True,
                sim_require_nnan=True,
                nc=nc,
            )
            return tuple(outs)

        devices = jax.devices()[:N_CORES]
        mesh = Mesh(np.asarray(devices), ("core",))
        in_specs = (PartitionSpec("core"),) * (n_params + n_outs)
        out_specs = (PartitionSpec("core"),) * len(out_names)
        self.sharded = jax.jit(
            shard_map(
                _body, mesh=mesh, in_specs=in_specs, out_specs=out_specs,
                check_rep=False,
            ),
            donate_argnums=donate,
            keep_unused=True,
        )
        self.jax = jax

    def run(self, in_maps):
        np_in = [
            np.concatenate([np.asarray(m[name]) for m in in_maps], axis=0)
            for name in self.in_names
        ]
        zeros = [
            np.zeros((N_CORES * z.shape[0], *z.shape[1:]), z.dtype)
            for z in self.zero_outs
        ]
        out_arrs = self.jax.block_until_ready(self.sharded(*np_in, *zeros))
        return [
            {
                name: np.asarray(out_arrs[i]).reshape(
                    N_CORES, *self.out_avals[i].shape
                )[c]
                for i, name in enumerate(self.out_names)
            }
            for c in range(N_CORES)
        ]


def _get_program(k: int) -> _CompiledProgram:
    prog = _PROGRAM_CACHE.get(k)
    if prog is None:
        prog = _CompiledProgram(k)
        _PROGRAM_CACHE[k] = prog
    return prog


def kernel(x, source, Wq, bq, Wk, bk, Wv, bv, Wm, bm, k):
    global LAST_RESULTS
    k = int(k)
    x = np.asarray(x, dtype=np.float32)
    source = np.asarray(source, dtype=np.float32)
    prog = _get_program(k)
    in_maps = make_in_maps(x, source, Wq, bq, Wk, bk, Wv, bv, Wm)
    results = prog.run(in_maps)
    LAST_RESULTS = results
    out = np.zeros((B, D, N), dtype=np.float32)
    for c in range(N_CORES):
        out[c // 2] += results[c]["part"]
    out += np.asarray(bm, dtype=np.float32)[None, :, None]
    return out
